# revision 12
# baseline (speedup 1.0000x reference)
"""LSTM decoder kernel for Trainium2, 8 NeuronCores.

Strategy: data-parallel over batch (32 rows/core, no collectives).
Per-core matmuls are batch-major with 4x column tiling; the 4 col-tile
matmuls run concurrently on HW (measured 3.7x). x_proj is injected into
PSUM as two bf16 identity-stationary matmuls (hi + lo split, exact to
2^-18). Gates are split into two PSUM tiles so dependency tracking lets
the activations on bank0 (i, g) start while bank1 (f, o) is still
accumulating; i*g is computed in that window too. The post-bank1 chain
(sigmoid f -> c update -> tanh -> h -> transpose -> copy) is the step's
critical path: it runs on ACT/DVE with the h/tanh ops half-chunked so
the first transposed half of h unblocks the next step's matmuls early.
y(t-1) and the xp injection for t+1 run on the PE inside that window,
keeping the PE busy so it stays at full clock (HAM).

Layout (per core, batch b in [0,32), col-tile j in [0,4)):
  bank0 tile Ga[32j+b, off+f]: i at off 0, g at off 256 (f in [0,256))
  bank1 tile Gb: f at off 0, o at off 256
  c/h tiles [128,256]: [32j+b, f] = state[b, 256j+f]
  hT chunks k: (T1 if k even else T2)[:, 32*(k//2)+ :32]
"""
import numpy as np
import ml_dtypes

import concourse.bass as bass
import concourse.mybir as mybir
import concourse.tile as tile
from concourse import bacc
from concourse import bass_utils

B, H, O, T, NCORES = 256, 1024, 512, 128, 8
BL = B // NCORES          # 32 batch rows per core
BF16 = mybir.dt.bfloat16
F32 = mybir.dt.float32

_CACHE = {}

# gate-MM k emission order: t1 consumers (even k) first so the first
# matmuls of step t+1 only need the first transposed half of h(t)
KORD = (0, 2, 4, 6, 1, 3, 5, 7)


def _emit_gate_mms(nc, ga, gf, go, w_sb, t1, t2):
    """h @ W_hh.T for one step (bf16, col-tiled).

    Emission order: bank0 (i,g) as N=512 windows, then f-windows and
    o-windows (N=256 each, in their own PSUM banks) so sigmoid(f) and
    the cell update can run while the o matmuls still stream without a
    same-bank PE-write/ACT-read hazard."""
    for k in KORD:
        tt = t1 if k % 2 == 0 else t2
        stat = tt[:, 32 * (k // 2):32 * (k // 2) + 32]
        for j in range(4):
            nc.tensor.matmul(
                ga[32 * j:32 * (j + 1), :], stat,
                w_sb[k][:, 512 * j:512 * (j + 1)],
                start=False, stop=(k == KORD[-1]),
                tile_position=(0, 32 * j), skip_group_check=True)
    for off, dst in ((0, gf), (256, go)):   # f pass, then o pass
        for k in KORD:
            tt = t1 if k % 2 == 0 else t2
            stat = tt[:, 32 * (k // 2):32 * (k // 2) + 32]
            for j in range(4):
                nc.tensor.matmul(
                    dst[32 * j:32 * (j + 1), 0:256], stat,
                    w_sb[k][:, 2048 + 512 * j + off:2048 + 512 * j + off + 256],
                    start=False, stop=(k == KORD[-1]),
                    tile_position=(0, 32 * j), skip_group_check=True)


XP_LO = True   # inject the bf16 residual term (exactness) — costs 8 matmuls


def _emit_xp_mms(nc, ga, gf, go, eyeb, xph_sb, xpl_sb, close):
    """bf16 hi+lo x_proj injection; hi opens each accumulation window.
    Window split mirrors _emit_gate_mms (bank0 N=512, bank1 2x N=256)."""
    stat0 = eyeb[:, 0:32]
    for j in range(4):
        w = j * 512
        out = ga[32 * j:32 * (j + 1), :]
        nc.tensor.matmul(out, stat0, xph_sb[:, w:w + 512],
                         start=True, stop=(close and not XP_LO),
                         tile_position=(0, 32 * j), skip_group_check=True)
        if XP_LO:
            nc.tensor.matmul(out, stat0, xpl_sb[:, w:w + 512],
                             start=False, stop=close,
                             tile_position=(0, 32 * j), skip_group_check=True)
    stat1 = eyeb[:, 32:64]
    for off, dst in ((0, gf), (256, go)):
        for j in range(4):
            w = (4 + j) * 512 + off
            out = dst[32 * j:32 * (j + 1), 0:256]
            nc.tensor.matmul(out, stat1, xph_sb[:, w:w + 256],
                             start=True, stop=(close and not XP_LO),
                             tile_position=(0, 32 * j), skip_group_check=True)
            if XP_LO:
                nc.tensor.matmul(out, stat1, xpl_sb[:, w:w + 256],
                                 start=False, stop=close,
                                 tile_position=(0, 32 * j), skip_group_check=True)


def _emit_y_mms(nc, y_ps, wl_sb, t1, t2):
    for k in range(8):
        tt = t1 if k % 2 == 0 else t2
        stat = tt[:, 32 * (k // 2):32 * (k // 2) + 32]
        for j in range(4):
            nc.tensor.matmul(
                y_ps[32 * j:32 * (j + 1), 0:128],
                stat,
                wl_sb[:, 512 * k + 128 * j:512 * k + 128 * j + 128],
                start=(k == 0),
                stop=(k == 7 and j == 3),
                tile_position=(0, 32 * j),
                skip_group_check=True,
            )


def _build(steps=T):
    nc = bacc.Bacc("TRN2", target_bir_lowering=False, debug=False,
                   num_devices=NCORES)
    w_d = nc.dram_tensor("W", [128, 8 * 4096], BF16, kind="ExternalInput").ap()
    wl_d = nc.dram_tensor("Wl", [128, 4096], BF16, kind="ExternalInput").ap()
    xph_d = nc.dram_tensor("xph", [128, 4096], BF16, kind="ExternalInput").ap()
    xpl_d = nc.dram_tensor("xpl", [128, 4096], BF16, kind="ExternalInput").ap()
    eyeb_d = nc.dram_tensor("eyeb", [128, 128], BF16, kind="ExternalInput").ap()
    y_d = nc.dram_tensor("y", [T, 128, 128], F32, kind="ExternalOutput").ap()

    ACT = mybir.ActivationFunctionType
    mult = mybir.AluOpType.mult
    addop = mybir.AluOpType.add

    with tile.TileContext(nc) as tc:
        with tc.tile_pool(name="stat", bufs=1) as statp, \
             tc.tile_pool(name="sb", bufs=2) as sb, \
             tc.tile_pool(name="ps", bufs=2, space="PSUM") as ps:
            w_sb = []
            for k in range(8):
                wk = statp.tile([128, 4096], BF16, tag=f"W{k}")
                nc.sync.dma_start(wk[:], w_d[:, 4096 * k:4096 * (k + 1)])
                w_sb.append(wk)
            wl_sb = statp.tile([128, 4096], BF16, tag="Wl")
            nc.sync.dma_start(wl_sb[:], wl_d)
            xph_sb = statp.tile([128, 4096], BF16, tag="xph")
            nc.sync.dma_start(xph_sb[:], xph_d)
            xpl_sb = statp.tile([128, 4096], BF16, tag="xpl")
            nc.sync.dma_start(xpl_sb[:], xpl_d)
            eyeb = statp.tile([128, 128], BF16, tag="eyeb")
            nc.sync.dma_start(eyeb[:], eyeb_d)
            c_sb = statp.tile([128, 256], F32, tag="c")
            nc.gpsimd.memset(c_sb[:], 0.0)

            t1_prev = t2_prev = None
            ga_cur = ps.tile([128, 512], F32, tag="ga")
            gf_cur = ps.tile([128, 512], F32, tag="gf")
            go_cur = ps.tile([128, 512], F32, tag="go")
            _emit_xp_mms(nc, ga_cur, gf_cur, go_cur, eyeb, xph_sb, xpl_sb,
                         close=True)

            for t in range(steps):
                if t > 0:
                    _emit_gate_mms(nc, ga_cur, gf_cur, go_cur, w_sb,
                                   t1_prev, t2_prev)

                # ACT consumers of the gate windows are emitted directly
                # after the matmuls (before the shadow y/xp work) so their
                # wakeup semaphores ride the window-closing matmuls, not
                # later PE instructions.
                sig_i = sb.tile([128, 256], F32, tag="si")
                nc.scalar.activation(sig_i[:], ga_cur[:, 0:256], ACT.Sigmoid)
                gt = sb.tile([128, 256], F32, tag="gt")
                nc.scalar.activation(gt[:], ga_cur[:, 256:512], ACT.Tanh)
                sig_f = sb.tile([128, 256], F32, tag="sf")
                nc.scalar.activation(sig_f[:], gf_cur[:, 0:256], ACT.Sigmoid)
                sig_o = sb.tile([128, 256], F32, tag="so")
                nc.scalar.activation(sig_o[:, 0:128], go_cur[:, 0:128],
                                     ACT.Sigmoid)
                nc.scalar.activation(sig_o[:, 128:256], go_cur[:, 128:256],
                                     ACT.Sigmoid)
                tmp = sb.tile([128, 256], F32, tag="tmp")
                nc.vector.tensor_tensor(tmp[:], sig_i[:], gt[:], mult)

                # PE work that is ready now and fills the tail window
                if t > 0:
                    y_ps = ps.tile([128, 512], F32, tag="y", bufs=1)
                    _emit_y_mms(nc, y_ps, wl_sb, t1_prev, t2_prev)
                if t < steps - 1:
                    ga_next = ps.tile([128, 512], F32, tag="ga")
                    gf_next = ps.tile([128, 512], F32, tag="gf")
                    go_next = ps.tile([128, 512], F32, tag="go")
                    _emit_xp_mms(nc, ga_next, gf_next, go_next, eyeb,
                                 xph_sb, xpl_sb, close=False)

                th = sb.tile([128, 256], F32, tag="th")
                h_sb = sb.tile([128, 256], BF16, tag="h")
                tp = ps.tile([128, 1024], BF16, tag="tp", bufs=1)
                t1 = sb.tile([128, 128], BF16, tag="t1")
                t2 = sb.tile([128, 128], BF16, tag="t2")
                for half in range(2):
                    lo, hi = 128 * half, 128 * (half + 1)
                    nc.vector.tensor_tensor(c_sb[:, lo:hi], sig_f[:, lo:hi],
                                            c_sb[:, lo:hi], mult)
                    nc.vector.tensor_tensor(c_sb[:, lo:hi], c_sb[:, lo:hi],
                                            tmp[:, lo:hi], addop)
                    nc.scalar.activation(th[:, lo:hi], c_sb[:, lo:hi],
                                         ACT.Tanh)
                # DVE order: h0, t1-copy (critical), then h1, t2-copy
                nc.vector.tensor_tensor(h_sb[:, 0:128], sig_o[:, 0:128],
                                        th[:, 0:128], mult)
                nc.tensor.transpose(tp[:, 0:128], h_sb[:, 0:128], eyeb[:])
                nc.vector.tensor_copy(t1[:], tp[:, 0:128])
                nc.vector.tensor_tensor(h_sb[:, 128:256], sig_o[:, 128:256],
                                        th[:, 128:256], mult)
                nc.tensor.transpose(tp[:, 128:256], h_sb[:, 128:256], eyeb[:])
                nc.vector.tensor_copy(t2[:], tp[:, 128:256])

                if t > 0:
                    y_sb = sb.tile([128, 128], F32, tag="ysb")
                    nc.scalar.activation(y_sb[:], y_ps[:, 0:128], ACT.Copy)
                    nc.sync.dma_start(y_d[t - 1], y_sb[:])

                t1_prev, t2_prev = t1, t2
                if t < steps - 1:
                    ga_cur, gf_cur, go_cur = ga_next, gf_next, go_next

            y_ps = ps.tile([128, 512], F32, tag="y", bufs=1)
            _emit_y_mms(nc, y_ps, wl_sb, t1_prev, t2_prev)
            y_sb = sb.tile([128, 128], F32, tag="ysb")
            nc.scalar.activation(y_sb[:], y_ps[:, 0:128], ACT.Copy)
            nc.sync.dma_start(y_d[steps - 1], y_sb[:])

    nc.compile()
    return nc


def _colmap():
    """Map device gate-column w -> original gate column.

    Device layout: bank0 = {i, g}, bank1 = {f, o} (torch order i,f,g,o
    has original col bases i=0, f=1024, g=2048, o=3072)."""
    m = np.empty(4096, np.int64)
    ar = np.arange(256)
    for j in range(4):
        m[512 * j:512 * j + 256] = 0 * 1024 + 256 * j + ar          # i
        m[512 * j + 256:512 * (j + 1)] = 2 * 1024 + 256 * j + ar    # g
        m[2048 + 512 * j:2048 + 512 * j + 256] = 1 * 1024 + 256 * j + ar   # f
        m[2048 + 512 * j + 256:2048 + 512 * (j + 1)] = 3 * 1024 + 256 * j + ar  # o
    return m


def _prep_inputs(C, W_ih, W_hh, b_ih, b_hh, W_lin):
    xp = np.asarray(C, np.float32) @ np.asarray(W_ih, np.float32).T
    xp = xp + np.asarray(b_ih, np.float32) + np.asarray(b_hh, np.float32)
    cm = _colmap()
    w_perm = np.asarray(W_hh, np.float32).T[:, cm]
    w_dev = np.ascontiguousarray(
        w_perm.reshape(8, 128, 4096)
        .transpose(1, 0, 2).reshape(128, 8 * 4096)).astype(ml_dtypes.bfloat16)
    wl_dev = np.ascontiguousarray(
        np.asarray(W_lin, np.float32).T.reshape(8, 128, 512)
        .transpose(1, 0, 2).reshape(128, 4096)).astype(ml_dtypes.bfloat16)
    eyeb = np.eye(128, dtype=ml_dtypes.bfloat16)
    in_maps = []
    for c in range(NCORES):
        xpb = xp[BL * c:BL * (c + 1)][:, cm]   # [32, 4096] in device col order
        xp_c = np.zeros((128, 4096), np.float32)
        for bank in range(2):
            xp_c[32 * bank:32 * (bank + 1), 2048 * bank:2048 * (bank + 1)] = \
                xpb[:, 2048 * bank:2048 * (bank + 1)]
        xph = xp_c.astype(ml_dtypes.bfloat16)
        xpl = (xp_c - xph.astype(np.float32)).astype(ml_dtypes.bfloat16)
        in_maps.append({"W": w_dev, "Wl": wl_dev, "xph": xph, "xpl": xpl,
                        "eyeb": eyeb})
    return in_maps


def kernel(C, W_ih, W_hh, b_ih, b_hh, W_lin, b_lin, max_seq_len):
    assert int(max_seq_len) == T and C.shape == (B, H)
    if "nc" not in _CACHE:
        _CACHE["nc"] = _build()
    nc = _CACHE["nc"]
    in_maps = _prep_inputs(C, W_ih, W_hh, b_ih, b_hh, W_lin)
    try:
        res = bass_utils.run_bass_kernel_spmd(
            nc, in_maps, core_ids=list(range(NCORES)))
    except Exception:
        # transient NRT faults have been observed on this fabric; retry once
        res = bass_utils.run_bass_kernel_spmd(
            nc, in_maps, core_ids=list(range(NCORES)))
    out = np.empty((T, B, O), np.float32)
    blin = np.asarray(b_lin, np.float32)
    for c in range(NCORES):
        yc = res.results[c]["y"]          # [T, 128, 128]
        out[:, BL * c:BL * (c + 1), :] = (
            yc.reshape(T, 4, BL, 128).transpose(0, 2, 1, 3).reshape(T, BL, O)
            + blin)
    return out


# revision 15
# speedup vs baseline: 2.2829x; 2.2829x over previous
"""LSTM decoder kernel for Trainium2, 8 NeuronCores.

Strategy: data-parallel over batch (32 rows/core, no collectives).
Per-core matmuls are batch-major with 4x column tiling; the 4 col-tile
matmuls run concurrently on HW (measured 3.7x). x_proj is injected into
PSUM as two bf16 identity-stationary matmuls (hi + lo split, exact to
2^-18). Gates are split into two PSUM tiles so dependency tracking lets
the activations on bank0 (i, g) start while bank1 (f, o) is still
accumulating; i*g is computed in that window too. The post-bank1 chain
(sigmoid f -> c update -> tanh -> h -> transpose -> copy) is the step's
critical path: it runs on ACT/DVE with the h/tanh ops half-chunked so
the first transposed half of h unblocks the next step's matmuls early.
y(t-1) and the xp injection for t+1 run on the PE inside that window,
keeping the PE busy so it stays at full clock (HAM).

Layout (per core, batch b in [0,32), col-tile j in [0,4)):
  bank0 tile Ga[32j+b, off+f]: i at off 0, g at off 256 (f in [0,256))
  bank1 tile Gb: f at off 0, o at off 256
  c/h tiles [128,256]: [32j+b, f] = state[b, 256j+f]
  hT chunks k: (T1 if k even else T2)[:, 32*(k//2)+ :32]
"""
import numpy as np
import ml_dtypes

import concourse.bass as bass
import concourse.mybir as mybir
import concourse.tile as tile
from concourse import bacc
from concourse import bass_utils

B, H, O, T, NCORES = 256, 1024, 512, 128, 8
BL = B // NCORES          # 32 batch rows per core
BF16 = mybir.dt.bfloat16
F32 = mybir.dt.float32

_CACHE = {}

# gate-MM k emission order: t1 consumers (even k) first so the first
# matmuls of step t+1 only need the first transposed half of h(t)
KORD = (0, 2, 4, 6, 1, 3, 5, 7)


def _emit_ga_mms(nc, ga, w_sb, t1, t2):
    """bank0 (i,g) h @ W_hh.T matmuls, N=512 windows."""
    for k in KORD:
        tt = t1 if k % 2 == 0 else t2
        stat = tt[:, 32 * (k // 2):32 * (k // 2) + 32]
        for j in range(4):
            nc.tensor.matmul(
                ga[32 * j:32 * (j + 1), :], stat,
                w_sb[k][:, 512 * j:512 * (j + 1)],
                start=False, stop=(k == KORD[-1]),
                tile_position=(0, 32 * j), skip_group_check=True)


def _emit_fo_mms(nc, dst, off, w_sb, t1, t2):
    """One N=256 bank1 pass (f at off=0 into gf, o at off=256 into go)."""
    for k in KORD:
        tt = t1 if k % 2 == 0 else t2
        stat = tt[:, 32 * (k // 2):32 * (k // 2) + 32]
        for j in range(4):
            nc.tensor.matmul(
                dst[32 * j:32 * (j + 1), 0:256], stat,
                w_sb[k][:, 2048 + 512 * j + off:2048 + 512 * j + off + 256],
                start=False, stop=(k == KORD[-1]),
                tile_position=(0, 32 * j), skip_group_check=True)


XP_LO = True   # inject the bf16 residual term (exactness) — costs 8 matmuls


def _emit_xp_mms(nc, ga, gf, go, eyeb, xph_sb, xpl_sb, close):
    """bf16 hi+lo x_proj injection; hi opens each accumulation window.
    Window split mirrors _emit_gate_mms (bank0 N=512, bank1 2x N=256)."""
    stat0 = eyeb[:, 0:32]
    for j in range(4):
        w = j * 512
        out = ga[32 * j:32 * (j + 1), :]
        nc.tensor.matmul(out, stat0, xph_sb[:, w:w + 512],
                         start=True, stop=(close and not XP_LO),
                         tile_position=(0, 32 * j), skip_group_check=True)
        if XP_LO:
            nc.tensor.matmul(out, stat0, xpl_sb[:, w:w + 512],
                             start=False, stop=close,
                             tile_position=(0, 32 * j), skip_group_check=True)
    stat1 = eyeb[:, 32:64]
    for off, dst in ((0, gf), (256, go)):
        for j in range(4):
            w = (4 + j) * 512 + off
            out = dst[32 * j:32 * (j + 1), 0:256]
            nc.tensor.matmul(out, stat1, xph_sb[:, w:w + 256],
                             start=True, stop=(close and not XP_LO),
                             tile_position=(0, 32 * j), skip_group_check=True)
            if XP_LO:
                nc.tensor.matmul(out, stat1, xpl_sb[:, w:w + 256],
                                 start=False, stop=close,
                                 tile_position=(0, 32 * j), skip_group_check=True)


def _emit_y_mms(nc, y_ps, wl_sb, t1, t2):
    for k in range(8):
        tt = t1 if k % 2 == 0 else t2
        stat = tt[:, 32 * (k // 2):32 * (k // 2) + 32]
        for j in range(4):
            nc.tensor.matmul(
                y_ps[32 * j:32 * (j + 1), 0:128],
                stat,
                wl_sb[:, 512 * k + 128 * j:512 * k + 128 * j + 128],
                start=(k == 0),
                stop=(k == 7 and j == 3),
                tile_position=(0, 32 * j),
                skip_group_check=True,
            )


def _build(steps=T):
    nc = bacc.Bacc("TRN2", target_bir_lowering=False, debug=False,
                   num_devices=NCORES)
    w_d = nc.dram_tensor("W", [128, 8 * 4096], BF16, kind="ExternalInput").ap()
    wl_d = nc.dram_tensor("Wl", [128, 4096], BF16, kind="ExternalInput").ap()
    xph_d = nc.dram_tensor("xph", [128, 4096], BF16, kind="ExternalInput").ap()
    xpl_d = nc.dram_tensor("xpl", [128, 4096], BF16, kind="ExternalInput").ap()
    eyeb_d = nc.dram_tensor("eyeb", [128, 128], BF16, kind="ExternalInput").ap()
    y_d = nc.dram_tensor("y", [T, 128, 128], F32, kind="ExternalOutput").ap()

    ACT = mybir.ActivationFunctionType
    mult = mybir.AluOpType.mult
    addop = mybir.AluOpType.add

    with tile.TileContext(nc) as tc:
        with tc.tile_pool(name="stat", bufs=1) as statp, \
             tc.tile_pool(name="sb", bufs=2) as sb, \
             tc.tile_pool(name="ps", bufs=2, space="PSUM") as ps:
            w_sb = []
            for k in range(8):
                wk = statp.tile([128, 4096], BF16, tag=f"W{k}")
                nc.sync.dma_start(wk[:], w_d[:, 4096 * k:4096 * (k + 1)])
                w_sb.append(wk)
            wl_sb = statp.tile([128, 4096], BF16, tag="Wl")
            nc.sync.dma_start(wl_sb[:], wl_d)
            xph_sb = statp.tile([128, 4096], BF16, tag="xph")
            nc.sync.dma_start(xph_sb[:], xph_d)
            xpl_sb = statp.tile([128, 4096], BF16, tag="xpl")
            nc.sync.dma_start(xpl_sb[:], xpl_d)
            eyeb = statp.tile([128, 128], BF16, tag="eyeb")
            nc.sync.dma_start(eyeb[:], eyeb_d)
            c_sb = statp.tile([128, 256], F32, tag="c")
            nc.gpsimd.memset(c_sb[:], 0.0)

            t1_prev = t2_prev = None
            ga_cur = ps.tile([128, 512], F32, tag="ga")
            gf_cur = ps.tile([128, 512], F32, tag="gf")
            go_cur = ps.tile([128, 512], F32, tag="go")
            _emit_xp_mms(nc, ga_cur, gf_cur, go_cur, eyeb, xph_sb, xpl_sb,
                         close=True)

            for t in range(steps):
                # PE stream per step: [ga][f][y(t-1)][o][xp(t+1)][T1][T2].
                # y between the f and o passes gives sigmoid(f) and the DVE
                # cell chain a ~1.4us head start inside the burst, and the
                # o-close wakeup rides the xp matmuls that follow directly.
                if t > 0:
                    _emit_ga_mms(nc, ga_cur, w_sb, t1_prev, t2_prev)

                sig_i = sb.tile([128, 256], F32, tag="si")
                nc.scalar.activation(sig_i[:], ga_cur[:, 0:256], ACT.Sigmoid)
                gt = sb.tile([128, 256], F32, tag="gt")
                nc.scalar.activation(gt[:], ga_cur[:, 256:512], ACT.Tanh)
                tmp = sb.tile([128, 256], F32, tag="tmp")
                nc.vector.tensor_tensor(tmp[:], sig_i[:], gt[:], mult)

                if t > 0:
                    _emit_fo_mms(nc, gf_cur, 0, w_sb, t1_prev, t2_prev)
                sig_f = sb.tile([128, 256], F32, tag="sf")
                nc.scalar.activation(sig_f[:], gf_cur[:, 0:256], ACT.Sigmoid)

                if t > 0:
                    y_ps = ps.tile([128, 512], F32, tag="y", bufs=1)
                    _emit_y_mms(nc, y_ps, wl_sb, t1_prev, t2_prev)
                    _emit_fo_mms(nc, go_cur, 256, w_sb, t1_prev, t2_prev)

                if t < steps - 1:
                    ga_next = ps.tile([128, 512], F32, tag="ga")
                    gf_next = ps.tile([128, 512], F32, tag="gf")
                    go_next = ps.tile([128, 512], F32, tag="go")
                    _emit_xp_mms(nc, ga_next, gf_next, go_next, eyeb,
                                 xph_sb, xpl_sb, close=False)

                # th halves run before sig_o on ACT: the c chain finishes
                # inside the burst, while sig_o must wait for the o windows
                th = sb.tile([128, 256], F32, tag="th")
                h_sb = sb.tile([128, 256], BF16, tag="h")
                tp = ps.tile([128, 1024], BF16, tag="tp", bufs=1)
                t1 = sb.tile([128, 128], BF16, tag="t1")
                t2 = sb.tile([128, 128], BF16, tag="t2")
                for half in range(2):
                    lo, hi = 128 * half, 128 * (half + 1)
                    nc.vector.tensor_tensor(c_sb[:, lo:hi], sig_f[:, lo:hi],
                                            c_sb[:, lo:hi], mult)
                    nc.vector.tensor_tensor(c_sb[:, lo:hi], c_sb[:, lo:hi],
                                            tmp[:, lo:hi], addop)
                    nc.scalar.activation(th[:, lo:hi], c_sb[:, lo:hi],
                                         ACT.Tanh)
                sig_o = sb.tile([128, 256], F32, tag="so")
                nc.scalar.activation(sig_o[:, 0:128], go_cur[:, 0:128],
                                     ACT.Sigmoid)
                nc.scalar.activation(sig_o[:, 128:256], go_cur[:, 128:256],
                                     ACT.Sigmoid)
                # DVE order: h0, t1-copy (critical), then h1, t2-copy
                nc.vector.tensor_tensor(h_sb[:, 0:128], sig_o[:, 0:128],
                                        th[:, 0:128], mult)
                nc.tensor.transpose(tp[:, 0:128], h_sb[:, 0:128], eyeb[:])
                nc.vector.tensor_copy(t1[:], tp[:, 0:128])
                nc.vector.tensor_tensor(h_sb[:, 128:256], sig_o[:, 128:256],
                                        th[:, 128:256], mult)
                nc.tensor.transpose(tp[:, 128:256], h_sb[:, 128:256], eyeb[:])
                nc.vector.tensor_copy(t2[:], tp[:, 128:256])

                if t > 0:
                    y_sb = sb.tile([128, 128], F32, tag="ysb")
                    nc.scalar.activation(y_sb[:], y_ps[:, 0:128], ACT.Copy)
                    nc.sync.dma_start(y_d[t - 1], y_sb[:])

                t1_prev, t2_prev = t1, t2
                if t < steps - 1:
                    ga_cur, gf_cur, go_cur = ga_next, gf_next, go_next

            y_ps = ps.tile([128, 512], F32, tag="y", bufs=1)
            _emit_y_mms(nc, y_ps, wl_sb, t1_prev, t2_prev)
            y_sb = sb.tile([128, 128], F32, tag="ysb")
            nc.scalar.activation(y_sb[:], y_ps[:, 0:128], ACT.Copy)
            nc.sync.dma_start(y_d[steps - 1], y_sb[:])

    nc.compile()
    return nc


def _colmap():
    """Map device gate-column w -> original gate column.

    Device layout: bank0 = {i, g}, bank1 = {f, o} (torch order i,f,g,o
    has original col bases i=0, f=1024, g=2048, o=3072)."""
    m = np.empty(4096, np.int64)
    ar = np.arange(256)
    for j in range(4):
        m[512 * j:512 * j + 256] = 0 * 1024 + 256 * j + ar          # i
        m[512 * j + 256:512 * (j + 1)] = 2 * 1024 + 256 * j + ar    # g
        m[2048 + 512 * j:2048 + 512 * j + 256] = 1 * 1024 + 256 * j + ar   # f
        m[2048 + 512 * j + 256:2048 + 512 * (j + 1)] = 3 * 1024 + 256 * j + ar  # o
    return m


def _prep_inputs(C, W_ih, W_hh, b_ih, b_hh, W_lin):
    xp = np.asarray(C, np.float32) @ np.asarray(W_ih, np.float32).T
    xp = xp + np.asarray(b_ih, np.float32) + np.asarray(b_hh, np.float32)
    cm = _colmap()
    w_perm = np.asarray(W_hh, np.float32).T[:, cm]
    w_dev = np.ascontiguousarray(
        w_perm.reshape(8, 128, 4096)
        .transpose(1, 0, 2).reshape(128, 8 * 4096)).astype(ml_dtypes.bfloat16)
    wl_dev = np.ascontiguousarray(
        np.asarray(W_lin, np.float32).T.reshape(8, 128, 512)
        .transpose(1, 0, 2).reshape(128, 4096)).astype(ml_dtypes.bfloat16)
    eyeb = np.eye(128, dtype=ml_dtypes.bfloat16)
    in_maps = []
    for c in range(NCORES):
        xpb = xp[BL * c:BL * (c + 1)][:, cm]   # [32, 4096] in device col order
        xp_c = np.zeros((128, 4096), np.float32)
        for bank in range(2):
            xp_c[32 * bank:32 * (bank + 1), 2048 * bank:2048 * (bank + 1)] = \
                xpb[:, 2048 * bank:2048 * (bank + 1)]
        xph = xp_c.astype(ml_dtypes.bfloat16)
        xpl = (xp_c - xph.astype(np.float32)).astype(ml_dtypes.bfloat16)
        in_maps.append({"W": w_dev, "Wl": wl_dev, "xph": xph, "xpl": xpl,
                        "eyeb": eyeb})
    return in_maps


def kernel(C, W_ih, W_hh, b_ih, b_hh, W_lin, b_lin, max_seq_len):
    assert int(max_seq_len) == T and C.shape == (B, H)
    if "nc" not in _CACHE:
        _CACHE["nc"] = _build()
    nc = _CACHE["nc"]
    in_maps = _prep_inputs(C, W_ih, W_hh, b_ih, b_hh, W_lin)
    try:
        res = bass_utils.run_bass_kernel_spmd(
            nc, in_maps, core_ids=list(range(NCORES)))
    except Exception:
        # transient NRT faults have been observed on this fabric; retry once
        res = bass_utils.run_bass_kernel_spmd(
            nc, in_maps, core_ids=list(range(NCORES)))
    out = np.empty((T, B, O), np.float32)
    blin = np.asarray(b_lin, np.float32)
    for c in range(NCORES):
        yc = res.results[c]["y"]          # [T, 128, 128]
        out[:, BL * c:BL * (c + 1), :] = (
            yc.reshape(T, 4, BL, 128).transpose(0, 2, 1, 3).reshape(T, BL, O)
            + blin)
    return out
